# revision 1
# baseline (speedup 1.0000x reference)
"""Trainium2 Bass kernel for nn_DenseNetCmaxGatedB2 (gated pooling block).

Computation (per batch, per channel, depthwise):
  out = maxpool3x3s2(x) * (dwconv_s2(x, maxgate) + mb)
  g0  = sigmoid(dwconv_s2(x, pgates[...,0]) + gbs[:,0])
  n0  = g0*p0 + (1-g0)*p1           p_k = dwconv_s2(x, pconvs[...,k]) + pbs[:,k]
  g1  = sigmoid(dwconv_s2(x, pgates[...,2]) + gbs[:,1])
  n1  = g1*p2 + (1-g1)*p3
  g   = sigmoid(dwconv_s1(n0, pgates[...,2]) + gbs[:,2])
  out = out + n0*g + n1*(1-g)

Sharding: pure data parallel over batch (16 -> 2 per core, 8 cores).

Layout: channels on SBUF partitions (128 per plane; 2 batches x 2
channel-blocks = 4 planes per core).  For bf16 compute, x is
deinterleaved once per plane into even/odd row x col parity planes so
every tap reads with unit stride (required for the DVE 2x/4x perf
modes).  Each stride-2 conv tap is a tensor_scalar product (4x bf16
mode, per-channel weight as the per-partition fp32 scalar) plus a
tensor_tensor accumulate (2x mode) — scalar_tensor_tensor would fuse
both but only has a 1x DVE uop, which measures slower.
TensorScalarPtr / TensorTensor are not legal Pool-engine opcodes on
TRN2 (walrus ISA check), so all elementwise compute runs on VectorE;
ScalarE does the deinterleave, the first tap of each conv (fused
scale+bias via activation Identity) and the sigmoids.

Measured (8-core SPMD, per-core HW time via For_i repeat-loop wall
deltas): ~1.31-1.41 ms with all products on DVE; rel err (absmax-
scaled) ~1.1e-2 vs fp32 ref.  Final version alternates tap products
between ScalarE and DVE (cost model: 937 us vs 1093 us) — numerically
identical ops, only engine placement differs.
"""

import contextlib
import sys

sys.path.insert(0, "/opt/trn_rl_repo")

import numpy as np

import concourse.bass as bass  # noqa: E402,F401
import concourse.mybir as mybir  # noqa: E402
from concourse import bacc  # noqa: E402
from concourse.tile import TileContext  # noqa: E402
from concourse.bass_utils import run_bass_kernel_spmd  # noqa: E402

N_CORES = 8
B, C, H = 16, 256, 128
HO = H // 2
BS = B // N_CORES  # batches per core
F32 = mybir.dt.float32
BF16 = mybir.dt.bfloat16
AF = mybir.ActivationFunctionType
OP = mybir.AluOpType

# (di, dj) tap order; (1,1) handled by ScalarE with fused scale+bias.
TAPS8 = [(0, 0), (0, 1), (0, 2), (1, 0), (1, 2), (2, 0), (2, 1), (2, 2)]


def _build(dt, reps=1):
    """Build the SPMD program for one core (2 batches, full channels).

    reps>1 wraps the per-plane pipeline in a hardware loop recomputing
    the same outputs; used only for wall-clock timing."""
    nc = bacc.Bacc("TRN2", target_bir_lowering=False, debug=False, num_devices=N_CORES)

    x_d = nc.dram_tensor("x", [BS, C, H * H], F32, kind="ExternalInput")
    mg_d = nc.dram_tensor("maxgate", [C, 9], F32, kind="ExternalInput")
    mb_d = nc.dram_tensor("mb", [C, 1], F32, kind="ExternalInput")
    pc_d = nc.dram_tensor("pconvs", [C, 36], F32, kind="ExternalInput")
    pb_d = nc.dram_tensor("pbs", [C, 4], F32, kind="ExternalInput")
    pg_d = nc.dram_tensor("pgates", [C, 27], F32, kind="ExternalInput")
    gb_d = nc.dram_tensor("gbs", [C, 3], F32, kind="ExternalInput")
    out_d = nc.dram_tensor("out", [BS, C, HO * HO], F32, kind="ExternalOutput")

    bf = dt == BF16
    V = nc.vector

    with TileContext(nc) as tc:
        with contextlib.ExitStack() as ctx:
            wp = ctx.enter_context(tc.tile_pool(name="w", bufs=1))
            xp = ctx.enter_context(tc.tile_pool(name="xp", bufs=1))
            pp = ctx.enter_context(tc.tile_pool(name="pp", bufs=2))
            ppz = ctx.enter_context(tc.tile_pool(name="ppz", bufs=1))
            ap = ctx.enter_context(tc.tile_pool(name="ap", bufs=1))
            op_ = ctx.enter_context(tc.tile_pool(name="op", bufs=2))

            # ---- weights / biases (fp32 per-partition scalars), per cblock
            W = []
            for cb in range(2):
                sl = slice(cb * 128, (cb + 1) * 128)
                wmg = wp.tile([128, 9], F32, tag=f"wmg{cb}")
                wpc = wp.tile([128, 36], F32, tag=f"wpc{cb}")
                wpg = wp.tile([128, 27], F32, tag=f"wpg{cb}")
                bmb = wp.tile([128, 1], F32, tag=f"bmb{cb}")
                bpb = wp.tile([128, 4], F32, tag=f"bpb{cb}")
                bgb = wp.tile([128, 3], F32, tag=f"bgb{cb}")
                nc.sync.dma_start(wmg[:], mg_d[sl, :])
                nc.sync.dma_start(wpc[:], pc_d[sl, :])
                nc.sync.dma_start(wpg[:], pg_d[sl, :])
                nc.sync.dma_start(bmb[:], mb_d[sl, :])
                nc.sync.dma_start(bpb[:], pb_d[sl, :])
                nc.sync.dma_start(bgb[:], gb_d[sl, :])

                def s(t, i):
                    return t[:, i : i + 1]

                def mk(wt, stride_, k):
                    return lambda di, dj, wt=wt, stride_=stride_, k=k: s(
                        wt, (di * 3 + dj) * stride_ + k
                    )

                W.append(
                    dict(
                        cm=(mk(wmg, 1, 0), s(bmb, 0)),
                        g0=(mk(wpg, 3, 0), s(bgb, 0)),
                        p0=(mk(wpc, 4, 0), s(bpb, 0)),
                        p1=(mk(wpc, 4, 1), s(bpb, 1)),
                        g1=(mk(wpg, 3, 2), s(bgb, 1)),
                        p2=(mk(wpc, 4, 2), s(bpb, 2)),
                        p3=(mk(wpc, 4, 3), s(bpb, 3)),
                        nd=(mk(wpg, 3, 2), s(bgb, 2)),
                    )
                )

            tmp_pool = ctx.enter_context(tc.tile_pool(name="tmp", bufs=2))

            def conv_s2(acc3, planes, wfn, bias):
                """Stride-2 3x3 depthwise conv into acc3 [128,64,64].

                scalar_tensor_tensor only has a 1x DVE uop, so instead each
                tap is a tensor_scalar product (4x mode in bf16) plus a
                tensor_tensor accumulate (2x mode) — ~35% fewer DVE cycles
                than the 1x fused MAC."""
                nc.scalar.activation(
                    acc3, planes["ee"][:], AF.Identity, bias=bias, scale=wfn(1, 1)
                )
                for di, dj in TAPS8:
                    rsel = "e" if di == 1 else "o"
                    csel = {0: "z", 1: "e", 2: "o"}[dj]
                    p = planes[rsel + csel]
                    i0 = 1 if di == 0 else 0
                    pin = p[:, 0 : 64 - i0, 0:64]
                    po = acc3[:, i0:64, :]
                    t = tmp_pool.tile([128, 64, 64], dt, tag="t", bufs=3, name="t")
                    tv = t[:, 0 : 64 - i0, :]
                    # alternate products between ScalarE (otherwise mostly
                    # idle) and DVE tensor_scalar (4x bf16); adds stay on DVE
                    if (di + dj) % 2 == 0:
                        nc.scalar.mul(tv, pin, wfn(di, dj))
                    else:
                        V.tensor_scalar(tv, pin, wfn(di, dj), None, OP.mult)
                    V.tensor_tensor(po, po, tv, OP.add)

            def conv_s2_strided(acc3, xv, wfn, bias):
                """fp32 path: taps read x [128,128,128] directly (strided)."""
                nc.scalar.activation(
                    acc3, xv[:, 0:128:2, 0:128:2], AF.Identity, bias=bias,
                    scale=wfn(1, 1),
                )
                for di, dj in TAPS8:
                    i0 = 1 if di == 0 else 0
                    j0 = 1 if dj == 0 else 0
                    r0 = di - 1 + 2 * i0
                    c0 = dj - 1 + 2 * j0
                    pin = xv[:, r0:128:2, c0:128:2][:, 0 : 64 - i0, 0 : 64 - j0]
                    po = acc3[:, i0:64, j0:64]
                    V.scalar_tensor_tensor(po, pin, wfn(di, dj), po, OP.mult, OP.add)

            def plane(b, cb):
                sl = slice(cb * 128, (cb + 1) * 128)
                w = W[cb]

                X = xp.tile([128, H * H], dt, tag="X", name="X")
                if bf:
                    nc.gpsimd.dma_start(X[:], x_d[b, sl, :])  # casts f32->bf16
                else:
                    nc.sync.dma_start(X[:], x_d[b, sl, :])
                xv = X[:].rearrange("p (r c) -> p r c", r=H)

                planes = None
                if bf:
                    pee = pp.tile([128, 64, 64], dt, tag="pee", name="pee")
                    peo = pp.tile([128, 64, 64], dt, tag="peo", name="peo")
                    poe = pp.tile([128, 64, 64], dt, tag="poe", name="poe")
                    poo = pp.tile([128, 64, 64], dt, tag="poo", name="poo")
                    pez = ppz.tile([128, 64, 65], dt, tag="pez", name="pez")
                    poz = ppz.tile([128, 64, 65], dt, tag="poz", name="poz")
                    nc.scalar.copy(pee[:], xv[:, 0:128:2, 0:128:2])
                    nc.scalar.copy(peo[:], xv[:, 0:128:2, 1:128:2])
                    nc.scalar.copy(poe[:], xv[:, 1:128:2, 0:128:2])
                    nc.scalar.copy(poo[:], xv[:, 1:128:2, 1:128:2])
                    nc.gpsimd.memset(pez[:, :, 0:1], 0)
                    nc.gpsimd.memset(poz[:, :, 0:1], 0)
                    nc.scalar.copy(pez[:, :, 1:65], xv[:, 0:128:2, 1:128:2])
                    nc.scalar.copy(poz[:, :, 1:65], xv[:, 1:128:2, 1:128:2])
                    planes = dict(ee=pee, eo=peo, oe=poe, oo=poo, ez=pez, oz=poz)

                def conv(acc3, key):
                    wfn, bias = w[key]
                    if bf:
                        conv_s2(acc3, planes, wfn, bias)
                    else:
                        conv_s2_strided(acc3, xv, wfn, bias)

                cm = ap.tile([128, 64, 64], dt, tag="A", name="cm")
                conv(cm[:], "cm")

                # maxpool via tensor_tensor max chain
                mp = ap.tile([128, 64, 64], dt, tag="B", name="mp")
                m3 = mp[:]
                rest = [(0, 0), (0, 1), (0, 2), (1, 0), (2, 0), (2, 1), (2, 2)]
                if bf:
                    V.tensor_tensor(m3, planes["ee"][:], planes["eo"][:], OP.max)
                    for di, dj in rest:
                        rsel = "e" if di == 1 else "o"
                        csel = {0: "o", 1: "e", 2: "o"}[dj]
                        p = planes[rsel + csel]
                        i0 = 1 if di == 0 else 0
                        j0 = 1 if dj == 0 else 0
                        pin = p[:, 0 : 64 - i0, 0 : 64 - j0]
                        po = m3[:, i0:64, j0:64]
                        V.tensor_tensor(po, po, pin, OP.max)
                else:
                    V.tensor_tensor(
                        m3, xv[:, 0:128:2, 0:128:2], xv[:, 0:128:2, 1:128:2], OP.max
                    )
                    for di, dj in rest:
                        i0 = 1 if di == 0 else 0
                        j0 = 1 if dj == 0 else 0
                        r0 = di - 1 + 2 * i0
                        c0 = dj - 1 + 2 * j0
                        pin = xv[:, r0:128:2, c0:128:2][:, 0 : 64 - i0, 0 : 64 - j0]
                        po = m3[:, i0:64, j0:64]
                        V.tensor_tensor(po, po, pin, OP.max)

                # mpcm = maxpool * cm   (keep in B)
                V.tensor_tensor(m3, m3, cm[:], OP.mult)

                g0 = ap.tile([128, 64, 64], dt, tag="A2", name="g0")
                conv(g0[:], "g0")
                nc.scalar.activation(g0[:], g0[:], AF.Sigmoid)

                p0 = ap.tile([128, 64, 64], dt, tag="C", name="p0")
                conv(p0[:], "p0")
                p1 = ap.tile([128, 64, 64], dt, tag="D", name="p1")
                conv(p1[:], "p1")

                # n0 = p1 + g0*(p0-p1), stored zero-padded [64,66]
                n0z = ap.tile([128, 64, 66], dt, tag="E", name="n0z")
                V.tensor_tensor(p0[:], p0[:], p1[:], OP.subtract)
                V.tensor_tensor(p0[:], p0[:], g0[:], OP.mult)
                nc.gpsimd.memset(n0z[:, :, 0:1], 0)
                nc.gpsimd.memset(n0z[:, :, 65:66], 0)
                n0 = n0z[:, :, 1:65]
                V.tensor_tensor(n0, p0[:], p1[:], OP.add)

                g1 = ap.tile([128, 64, 64], dt, tag="A2", name="g1")
                conv(g1[:], "g1")
                nc.scalar.activation(g1[:], g1[:], AF.Sigmoid)
                p2 = ap.tile([128, 64, 64], dt, tag="C", name="p2")
                conv(p2[:], "p2")
                p3 = ap.tile([128, 64, 64], dt, tag="D", name="p3")
                conv(p3[:], "p3")

                V.tensor_tensor(p2[:], p2[:], p3[:], OP.subtract)
                V.tensor_tensor(p2[:], p2[:], g1[:], OP.mult)
                V.tensor_tensor(p2[:], p2[:], p3[:], OP.add)
                n1 = p2  # tag C

                # node-stage gate: stride-1 conv over padded n0
                gc = ap.tile([128, 64, 64], dt, tag="A2", name="gc")
                wfn, bias = w["nd"]
                nc.scalar.activation(
                    gc[:], n0z[:, 0:64, 1:65], AF.Identity, bias=bias, scale=wfn(1, 1)
                )
                for di, dj in TAPS8:
                    i0 = 1 if di == 0 else 0
                    i1 = 1 if di == 2 else 0
                    r0n = di - 1 + i0
                    pin = n0z[:, r0n : r0n + 64 - i0 - i1, dj : dj + 64]
                    po = gc[:, i0 : 64 - i1, :]
                    t = tmp_pool.tile([128, 64, 64], dt, tag="t", bufs=3, name="t")
                    tv = t[:, 0 : 64 - i0 - i1, :]
                    if (di + dj) % 2 == 0:
                        nc.scalar.mul(tv, pin, wfn(di, dj))
                    else:
                        V.tensor_scalar(tv, pin, wfn(di, dj), None, OP.mult)
                    V.tensor_tensor(po, po, tv, OP.add)
                nc.scalar.activation(gc[:], gc[:], AF.Sigmoid)

                # out = mpcm + n1 + g*(n0-n1)
                o = op_.tile([128, 64, 64], dt, tag="O", name="o")
                V.tensor_tensor(o[:], n0, n1[:], OP.subtract)
                V.tensor_tensor(o[:], o[:], gc[:], OP.mult)
                V.tensor_tensor(o[:], o[:], n1[:], OP.add)
                V.tensor_tensor(o[:], o[:], m3, OP.add)

                oflat = o[:].rearrange("p a b -> p (a b)")
                if bf:
                    nc.gpsimd.dma_start(out_d[b, sl, :], oflat)  # cast back
                else:
                    nc.sync.dma_start(out_d[b, sl, :], oflat)

            rep_ctx = tc.For_i(0, reps, 1) if reps > 1 else contextlib.nullcontext()
            with rep_ctx:
                for b in range(BS):
                    for cb in range(2):
                        plane(b, cb)

    nc.compile()
    return nc


_NC_CACHE = {}


def _get_nc(dt, reps=1):
    key = (str(dt), reps)
    if key not in _NC_CACHE:
        _NC_CACHE[key] = _build(dt, reps)
    return _NC_CACHE[key]


def _in_maps(x, maxgate, mb, pconvs, pbs, pgates, gbs):
    x = np.ascontiguousarray(np.asarray(x, np.float32))
    maps = []
    for i in range(N_CORES):
        maps.append(
            dict(
                x=x[i * BS : (i + 1) * BS].reshape(BS, C, H * H),
                maxgate=np.asarray(maxgate, np.float32).reshape(C, 9),
                mb=np.asarray(mb, np.float32).reshape(C, 1),
                pconvs=np.asarray(pconvs, np.float32).reshape(C, 36),
                pbs=np.asarray(pbs, np.float32).reshape(C, 4),
                pgates=np.asarray(pgates, np.float32).reshape(C, 27),
                gbs=np.asarray(gbs, np.float32).reshape(C, 3),
            )
        )
    return maps


def kernel(x, maxgate, mb, pconvs, pbs, pgates, gbs):
    nc = _get_nc(BF16)
    maps = _in_maps(x, maxgate, mb, pconvs, pbs, pgates, gbs)
    res = run_bass_kernel_spmd(nc, maps, list(range(N_CORES)))
    return np.concatenate(
        [r["out"].reshape(BS, C, HO, HO) for r in res.results], axis=0
    )



# revision 2
# speedup vs baseline: 3.4086x; 3.4086x over previous
"""Trainium2 Bass kernel for nn_DenseNetCmaxGatedB2 (gated pooling block).

Rewritten from the DVE/ACT-only baseline to use all engines:
  out = maxpool3x3s2(x) * (dwconv_s2(x, maxgate) + mb) + n1 + g*(n0 - n1)
  n0  = p1 + g0*d01;  n1 = p3 + g1*d23    (d01/d23 = difference-weight convs)
  g0/g1/g = sigmoid(conv + bias);  node conv g is stride-1 on n0.

Knobs:
  PE_CONVS : which stride-2 convs run on the TensorE (diagonal matmuls)
  TILE4    : use 4x (32x32) diagonal tile_position matmuls (hides LDWEIGHTS)
  cm + any conv not in PE_CONVS runs on DVE (tensor_scalar 4x products +
  tensor_tensor 2x adds, products alternating with ACT).
"""

import contextlib
import sys

sys.path.insert(0, "/opt/trn_rl_repo")

import numpy as np

import concourse.bass as bass  # noqa: E402,F401
import concourse.mybir as mybir  # noqa: E402
from concourse import bacc  # noqa: E402
from concourse.tile import TileContext  # noqa: E402
from concourse.bass_utils import run_bass_kernel_spmd  # noqa: E402

N_CORES = 8
B, C, H = 16, 256, 128
HO = H // 2
BS = B // N_CORES
F32 = mybir.dt.float32
BF16 = mybir.dt.bfloat16
AF = mybir.ActivationFunctionType
OP = mybir.AluOpType

ALL_CONVS = ["g0", "d01", "p1", "g1", "d23", "p3"]
PE_CONVS = ["g0", "d01", "p1", "g1", "d23"]  # p3 -> DVE
PROD_ACT = {0, 2, 6, 8}  # tap indices whose products go to ACT
TILE4 = True
BIAS_KEYS = ["mb", "g0", "d01", "p1", "g1", "d23", "p3", "gc"]
def _sc_convs():
    return ["mb"] + [k for k in ALL_CONVS if k not in PE_CONVS]
TAPS9 = [(di, dj) for di in range(3) for dj in range(3)]


def _build(reps=1):
    SC_CONVS = _sc_convs()
    npe = len(PE_CONVS)
    nsc = len(SC_CONVS)
    nc = bacc.Bacc("TRN2", target_bir_lowering=False, debug=False, num_devices=N_CORES)

    x_d = nc.dram_tensor("x", [BS, C, H * H], F32, kind="ExternalInput")
    wdiag_d = nc.dram_tensor(
        "wdiag", [128, 2 * npe * 9 * 128], F32, kind="ExternalInput"
    )
    wsc_d = nc.dram_tensor("wsc", [C, nsc * 9], F32, kind="ExternalInput")
    bias_d = nc.dram_tensor("biases", [C, len(BIAS_KEYS)], F32, kind="ExternalInput")
    out_d = nc.dram_tensor("out", [BS, C, HO * HO], F32, kind="ExternalOutput")

    V = nc.vector
    S = nc.scalar
    G = nc.gpsimd

    with TileContext(nc) as tc:
        with contextlib.ExitStack() as ctx:
            wp = ctx.enter_context(tc.tile_pool(name="w", bufs=1))
            xhp = ctx.enter_context(tc.tile_pool(name="xh", bufs=2))
            pp = ctx.enter_context(tc.tile_pool(name="pp", bufs=1))
            evp = ctx.enter_context(tc.tile_pool(name="ev", bufs=2))
            n0p = ctx.enter_context(tc.tile_pool(name="n0p", bufs=2))
            mid = ctx.enter_context(tc.tile_pool(name="mid", bufs=1))
            tmp = ctx.enter_context(tc.tile_pool(name="tmp", bufs=2))
            ps = ctx.enter_context(tc.tile_pool(name="ps", bufs=2, space="PSUM"))

            # ---- weights (resident)
            w9 = wp.tile([128, 2, npe, 9, 128], BF16, tag="w9")
            G.dma_start(w9[:].rearrange("p a b c d -> p (a b c d)"), wdiag_d[:, :])
            wsc = wp.tile([128, 2, nsc, 9], F32, tag="wsc")
            bb = wp.tile([128, 2, len(BIAS_KEYS)], F32, tag="bb")
            for cb in range(2):
                sl = slice(cb * 128, (cb + 1) * 128)
                nc.sync.dma_start(
                    wsc[:, cb, :, :].rearrange("p a b -> p (a b)"), wsc_d[sl, :]
                )
                nc.sync.dma_start(bb[:, cb, :], bias_d[sl, :])

            def bias_ap(cb, key):
                k = BIAS_KEYS.index(key)
                return bb[:, cb, k : k + 1]

            state = {}

            def stage_a(b, cb):
                sl = slice(cb * 128, (cb + 1) * 128)

                ee = pp.tile([128, 64, 64], BF16, tag="ee", name="ee")
                eo = pp.tile([128, 64, 64], BF16, tag="eo", name="eo")
                oe = pp.tile([128, 64, 64], BF16, tag="oe", name="oe")
                oo = pp.tile([128, 64, 64], BF16, tag="oo", name="oo")
                ez = pp.tile([128, 64, 66], BF16, tag="ez", name="ez")
                oz = pp.tile([128, 64, 66], BF16, tag="oz", name="oz")
                G.memset(ez[:, :, 0:1], 0)
                G.memset(oz[:, :, 0:1], 0)

                for h in range(4):
                    Xh = xhp.tile([128, 32, 128], BF16, tag="Xh", name="Xh")
                    G.dma_start(
                        Xh[:].rearrange("p a b -> p (a b)"),
                        x_d[b, sl, h * 4096 : (h + 1) * 4096],
                    )
                    hs = slice(16 * h, 16 * h + 16)
                    S.copy(ee[:, hs, :], Xh[:, 0:32:2, 0:128:2])
                    S.copy(eo[:, hs, :], Xh[:, 0:32:2, 1:128:2])
                    S.copy(oe[:, hs, :], Xh[:, 1:32:2, 0:128:2])
                    S.copy(oo[:, hs, :], Xh[:, 1:32:2, 1:128:2])
                    G.tensor_copy(ez[:, hs, 1:65], Xh[:, 0:32:2, 1:128:2])
                    G.tensor_copy(oz[:, hs, 1:65], Xh[:, 1:32:2, 1:128:2])

                def plane_view(di, dj):
                    if di == 1:
                        return {0: ez, 1: ee, 2: eo}[dj]
                    return {0: oz, 1: oe, 2: oo}[dj]

                def ev_tile(key):
                    if key in ("g0", "g1"):
                        return evp.tile([128, 64, 64], BF16, tag="g", name=key)
                    if key in ("d01", "d23"):
                        return evp.tile([128, 64, 64], BF16, tag="d", name=key)
                    if key == "p1":
                        return evp.tile(
                            [128, 64, 64], BF16, tag="p1", name=key, bufs=1
                        )
                    return evp.tile([128, 64, 64], BF16, tag="p3", name=key)

                ev_tiles = {}

                # ---- PE convs
                for key in PE_CONVS:
                    cvi = PE_CONVS.index(key)
                    dst = ev_tile(key)
                    ev_tiles[key] = dst
                    func = AF.Sigmoid if key in ("g0", "g1") else AF.Identity
                    for h in range(2):
                        acc = ps.tile([128, 2048], F32, tag="ph", name="ph")
                        for c in range(4):
                            r0c = 32 * h + 8 * c
                            for t, (di, dj) in enumerate(TAPS9):
                                pl = plane_view(di, dj)
                                ro = -1 if di == 0 else 0
                                r0, nr, o0 = r0c, 8, 0
                                if di == 0 and r0c == 0:
                                    r0, nr, o0 = 1, 7, 64
                                if TILE4:
                                    for g in range(4):
                                        gs = slice(32 * g, 32 * g + 32)
                                        nc.tensor.matmul(
                                            acc[gs, 512 * c + o0 : 512 * c + 512],
                                            w9[gs, cb, cvi, t, gs],
                                            pl[gs, r0 + ro : r0 + ro + nr, 0:64],
                                            start=(t == 0),
                                            stop=(t == 8),
                                            tile_position=(32 * g, 32 * g),
                                        )
                                else:
                                    nc.tensor.matmul(
                                        acc[:, 512 * c + o0 : 512 * c + 512],
                                        w9[:, cb, cvi, t, :],
                                        pl[:, r0 + ro : r0 + ro + nr, 0:64],
                                        start=(t == 0),
                                        stop=(t == 8),
                                    )
                        S.activation(
                            dst[:, 32 * h : 32 * h + 32, :],
                            acc[:].rearrange("p (r c) -> p r c", r=32),
                            func,
                            bias=bias_ap(cb, key),
                        )

                # ---- DVE convs (cm + any conv not on PE)
                def wsc_s(key, t):
                    k = _sc_convs().index(key)
                    return wsc[:, cb, k, t : t + 1]

                def dve_conv(key, dst, bias_key):
                    V.tensor_scalar(
                        dst[:], ee[:], wsc_s(key, 4), bias_ap(cb, bias_key),
                        OP.mult, OP.add,
                    )
                    for t, (di, dj) in enumerate(TAPS9):
                        if di == 1 and dj == 1:
                            continue
                        pl = plane_view(di, dj)
                        i0 = 1 if di == 0 else 0
                        pin = pl[:, 0 : 64 - i0, 0:64]
                        po = dst[:, i0:64, :]
                        t_ = tmp.tile([128, 64, 64], BF16, tag="t", bufs=2, name="t")
                        tv = t_[:, 0 : 64 - i0, :]
                        if t in PROD_ACT:
                            S.mul(tv, pin, wsc_s(key, t))
                        else:
                            V.tensor_scalar(tv, pin, wsc_s(key, t), None, OP.mult)
                        V.tensor_tensor(po, po, tv, OP.add)

                cm = mid.tile([128, 64, 64], BF16, tag="cm", name="cm", bufs=2)
                dve_conv("mb", cm, "mb")
                for key in ALL_CONVS:
                    if key in PE_CONVS:
                        continue
                    dst = ev_tile(key)
                    ev_tiles[key] = dst
                    dve_conv(key, dst, key)
                    if key in ("g0", "g1"):
                        S.activation(dst[:], dst[:], AF.Sigmoid)

                # ---- separable maxpool -> mp
                mp = mid.tile([128, 64, 64], BF16, tag="mp", name="mp")
                m1e = tmp.tile([128, 64, 64], BF16, tag="t", bufs=2, name="m1e")
                m1o = tmp.tile([128, 64, 64], BF16, tag="t", bufs=2, name="m1o")
                V.tensor_tensor(m1e[:], ee[:], eo[:], OP.max)
                V.tensor_tensor(m1o[:], oe[:], oo[:], OP.max)
                V.tensor_tensor(
                    m1e[:, :, 1:64], m1e[:, :, 1:64], ez[:, :, 1:64], OP.max
                )
                V.tensor_tensor(
                    m1o[:, :, 1:64], m1o[:, :, 1:64], oz[:, :, 1:64], OP.max
                )
                V.tensor_tensor(mp[:], m1e[:], m1o[:], OP.max)
                V.tensor_tensor(
                    mp[:, 1:64, :], mp[:, 1:64, :], m1o[:, 0:63, :], OP.max
                )

                V.tensor_tensor(cm[:], cm[:], mp[:], OP.mult)  # mpcm

                n0z = n0p.tile([128, 66, 68], BF16, tag="n0z", name="n0z")
                G.memset(n0z[:, 0:1, :], 0)
                G.memset(n0z[:, 65:66, :], 0)
                G.memset(n0z[:, 1:65, 1:2], 0)
                G.memset(n0z[:, 1:65, 66:67], 0)
                e01 = ev_tiles["d01"]
                V.tensor_tensor(e01[:], e01[:], ev_tiles["g0"][:], OP.mult)
                V.tensor_tensor(n0z[:, 1:65, 2:66], ev_tiles["p1"][:], e01[:], OP.add)
                e23 = ev_tiles["d23"]
                V.tensor_tensor(e23[:], e23[:], ev_tiles["g1"][:], OP.mult)
                n1 = ev_tiles["p3"]
                V.tensor_tensor(n1[:], n1[:], e23[:], OP.add)

                state[(b, cb)] = dict(n0z=n0z, n1=n1, mpcm=cm)

            def stage_b(b, cb):
                sl = slice(cb * 128, (cb + 1) * 128)
                st = state.pop((b, cb))
                n0z, n1, mpcm = st["n0z"], st["n1"], st["mpcm"]

                g1i = PE_CONVS.index("g1")
                gc = evp.tile([128, 64, 64], BF16, tag="g", name="gc")
                for h in range(2):
                    acc = ps.tile([128, 2048], F32, tag="ph", name="ph")
                    for c in range(4):
                        r0 = 32 * h + 8 * c
                        for t, (di, dj) in enumerate(TAPS9):
                            if TILE4:
                                for g in range(4):
                                    gs = slice(32 * g, 32 * g + 32)
                                    nc.tensor.matmul(
                                        acc[gs, 512 * c : 512 * c + 512],
                                        w9[gs, cb, g1i, t, gs],
                                        n0z[gs, r0 + di : r0 + di + 8, dj + 1 : dj + 65],
                                        start=(t == 0),
                                        stop=(t == 8),
                                        tile_position=(32 * g, 32 * g),
                                    )
                            else:
                                nc.tensor.matmul(
                                    acc[:, 512 * c : 512 * c + 512],
                                    w9[:, cb, g1i, t, :],
                                    n0z[:, r0 + di : r0 + di + 8, dj + 1 : dj + 65],
                                    start=(t == 0),
                                    stop=(t == 8),
                                )
                    S.activation(
                        gc[:, 32 * h : 32 * h + 32, :],
                        acc[:].rearrange("p (r c) -> p r c", r=32),
                        AF.Sigmoid,
                        bias=bias_ap(cb, "gc"),
                    )

                n0 = n0z[:, 1:65, 2:66]
                d = tmp.tile([128, 64, 64], BF16, tag="t", bufs=2, name="d")
                V.tensor_tensor(d[:], n0, n1[:], OP.subtract)
                V.tensor_tensor(d[:], d[:], gc[:], OP.mult)
                V.tensor_tensor(d[:], d[:], n1[:], OP.add)
                V.tensor_tensor(d[:], d[:], mpcm[:], OP.add)
                G.dma_start(out_d[b, sl, :], d[:].rearrange("p a b -> p (a b)"))

            planes = [(b, cb) for b in range(BS) for cb in range(2)]
            rep_ctx = tc.For_i(0, reps, 1) if reps > 1 else contextlib.nullcontext()
            with rep_ctx:
                stage_a(*planes[0])
                for i in range(1, len(planes)):
                    stage_a(*planes[i])
                    stage_b(*planes[i - 1])
                stage_b(*planes[-1])

    nc.compile()
    return nc


_NC_CACHE = {}


def _get_nc(reps=1):
    key = (tuple(PE_CONVS), TILE4, reps)
    if key not in _NC_CACHE:
        _NC_CACHE[key] = _build(reps)
    return _NC_CACHE[key]


def _prep_weights(maxgate, mb, pconvs, pbs, pgates, gbs):
    SC_CONVS = _sc_convs()
    npe = len(PE_CONVS)
    mg = np.asarray(maxgate, np.float32).reshape(C, 9)
    pc = np.asarray(pconvs, np.float32).reshape(C, 9, 4)
    pg = np.asarray(pgates, np.float32).reshape(C, 9, 3)
    pbs = np.asarray(pbs, np.float32)
    gbs = np.asarray(gbs, np.float32)
    mb = np.asarray(mb, np.float32).reshape(C)

    wconvs = {
        "mb": mg,
        "g0": pg[:, :, 0],
        "d01": pc[:, :, 0] - pc[:, :, 1],
        "p1": pc[:, :, 1],
        "g1": pg[:, :, 2],
        "d23": pc[:, :, 2] - pc[:, :, 3],
        "p3": pc[:, :, 3],
    }
    wd = np.zeros((128, 2, npe, 9, 128), np.float32)
    idx = np.arange(128)
    for cb in range(2):
        for cvi, key in enumerate(PE_CONVS):
            wd[idx, cb, cvi, :, idx] = wconvs[key][cb * 128 : (cb + 1) * 128, :]
    wsc = np.stack([wconvs[k] for k in SC_CONVS], axis=1)  # [C, nsc, 9]
    biases = np.stack(
        [
            mb,
            gbs[:, 0],
            pbs[:, 0] - pbs[:, 1],
            pbs[:, 1],
            gbs[:, 1],
            pbs[:, 2] - pbs[:, 3],
            pbs[:, 3],
            gbs[:, 2],
        ],
        axis=1,
    ).astype(np.float32)
    return (
        wd.reshape(128, 2 * npe * 9 * 128),
        wsc.reshape(C, len(SC_CONVS) * 9).astype(np.float32),
        biases,
    )


def _in_maps(x, maxgate, mb, pconvs, pbs, pgates, gbs):
    x = np.ascontiguousarray(np.asarray(x, np.float32))
    wd, wsc, biases = _prep_weights(maxgate, mb, pconvs, pbs, pgates, gbs)
    maps = []
    for i in range(N_CORES):
        maps.append(
            dict(
                x=x[i * BS : (i + 1) * BS].reshape(BS, C, H * H),
                wdiag=wd,
                wsc=wsc,
                biases=biases,
            )
        )
    return maps


def kernel(x, maxgate, mb, pconvs, pbs, pgates, gbs):
    nc = _get_nc(1)
    maps = _in_maps(x, maxgate, mb, pconvs, pbs, pgates, gbs)
    res = run_bass_kernel_spmd(nc, maps, list(range(N_CORES)))
    return np.concatenate(
        [r["out"].reshape(BS, C, HO, HO) for r in res.results], axis=0
    )
